# revision 4
# baseline (speedup 1.0000x reference)
"""Causal self-attention with RoPE, tensor-parallel over 8 NeuronCores.

Problem shapes: x [2, 2048, 2048], w_qkv [6144, 2048], w_out [2048, 2048],
H=16 heads, dh=128. Core c owns heads {2c, 2c+1}.

Per-core plan (all layouts chosen so no on-device transposes are needed
except V, which gets a cheap PE transpose):
  - host passes xT [B, D, L] plus per-core transposed weight shards
  - QKV^T projection: psum[dh, tok] = W^T-tile (stationary) x xT (moving)
  - RoPE applied during PSUM->SBUF copyback via crossed-base-partition
    DVE ops (half-swap without any shuffle instruction)
  - scores^T[k, q] = K^T-tile (stationary) x Q^T (moving); exp on ACT with
    the 1/sqrt(dh) scale folded in; causal mask = multiply by 0/1 tiles on
    the (only partially-valid) diagonal blocks; fully-masked blocks skipped
  - unnormalized out^T[dh, q] accumulates V-tile x E^T; the softmax
    denominator accumulates ones^T x E^T in the same pass
  - normalization: reciprocal of the [1, 512] sums row, broadcast across
    partitions with a K=1 matmul, multiplied in during the attention-out
    copyback
  - w_out partial[tok, e] = attnout^T-tile (stationary) x woT (moving),
    summed over this core's heads in PSUM; host sums the 8 partials

Matmuls run in float32r (full PE rate for free dim >= 256, ~1.5e-4 rel
err per K=128 contraction measured on hw).
"""

import numpy as np

import concourse.bass as bass
import concourse.mybir as mybir
import concourse.tile as tile
from concourse import bacc
from concourse.bass_utils import run_bass_kernel_spmd
from concourse.masks import make_identity

B, L, D, H = 2, 2048, 2048, 16
DH = D // H  # 128
NCORES = 8
HPC = H // NCORES  # heads per core
ROPE_BASE = 10000.0
SCALE = 1.0 / float(np.sqrt(np.float32(DH)))

TOKC = 256  # token chunk width in the QKV projection phase
NCHUNK = L // TOKC  # 8
QC = 512  # q chunk width in the attention phase
NQC = L // QC  # 4
KT = L // 128  # 16 k tiles per sequence
KD = D // 128  # 16 contraction chunks for the projections

F32 = mybir.dt.float32
F32R = mybir.dt.float32r
AF = mybir.ActivationFunctionType
ALU = mybir.AluOpType


def _body(nc, tc, aps):
    xt, wq, wk, wv, wo, cs, sn, mk, out = aps
    with (
        tc.tile_pool(name="const", bufs=1) as const,
        tc.tile_pool(name="xtp", bufs=2) as xtp,
        tc.tile_pool(name="qkv", bufs=1) as qkvp,
        tc.tile_pool(name="tmps", bufs=2) as tmps,
        tc.tile_pool(name="esb", bufs=2) as esbp,
        tc.tile_pool(name="bcp", bufs=1) as bcp,
        tc.tile_pool(name="attn", bufs=1) as attnp,
        tc.tile_pool(name="outp", bufs=3) as outp,
        tc.tile_pool(name="psA", bufs=3, space="PSUM") as psA,
        tc.tile_pool(name="psO", bufs=2, space="PSUM") as psO,
        tc.tile_pool(name="psS", bufs=1, space="PSUM") as psS,
        tc.tile_pool(name="psB", bufs=1, space="PSUM") as psB,
    ):
        # ---- constants ----
        wq_sb = const.tile([128, KD, HPC * DH], F32R, name="wq_sb")
        wk_sb = const.tile([128, KD, HPC * DH], F32R, name="wk_sb")
        wv_sb = const.tile([128, KD, HPC * DH], F32R, name="wv_sb")
        wo_sb = const.tile([128, HPC, D], F32R, name="wo_sb")
        cs_sb = const.tile([128, L], F32, name="cs_sb")
        sn_sb = const.tile([128, L], F32, name="sn_sb")
        mk_sb = const.tile([128, 4, QC], F32R, name="mk_sb")
        for dst, src in ((wq_sb, wq), (wk_sb, wk), (wv_sb, wv), (wo_sb, wo),
                         (cs_sb, cs), (sn_sb, sn), (mk_sb, mk)):
            nc.sync.dma_start(dst, src)
        ident = const.tile([128, 128], F32, name="ident")
        make_identity(nc, ident)
        ones_row = const.tile([1, 128], F32, name="ones_row")
        nc.vector.memset(ones_row, 1.0)
        ones_f32 = const.tile([128, 1], F32, name="ones_f32")
        nc.vector.memset(ones_f32, 1.0)
        ones_col = const.tile([128, 1], F32R, name="ones_col")
        nc.vector.tensor_copy(ones_col, ones_f32)

        for b in range(B):
            # ---- QKV projection + RoPE for batch b ----
            qrot = [qkvp.tile([128, L], F32R, name=f"qrot{h}") for h in range(HPC)]
            krot = [qkvp.tile([128, L], F32R, name=f"krot{h}") for h in range(HPC)]
            vnat = [qkvp.tile([128, KT, 128], F32R, name=f"vnat{h}")
                    for h in range(HPC)]
            for c in range(NCHUNK):
                c0 = c * TOKC
                xtile = xtp.tile([128, KD, TOKC], F32R, name="xtile")
                nc.sync.dma_start(
                    xtile, xt[b, :, c0:c0 + TOKC].rearrange("(ko p) n -> p ko n", p=128)
                )
                for w_sb, dsts, kind in (
                    (wq_sb, qrot, "q"), (wk_sb, krot, "k"), (wv_sb, vnat, "v")
                ):
                    for h in range(HPC):
                        ps = psA.tile([128, TOKC], F32, name="ps_proj", tag="psA")
                        for k in range(KD):
                            nc.tensor.matmul(
                                ps, w_sb[:, k, h * DH:(h + 1) * DH], xtile[:, k, :],
                                start=(k == 0), stop=(k == KD - 1),
                            )
                        if kind in ("q", "k"):
                            # RoPE copyback: dst = psum*cos + swap(psum)*sin_signed
                            t = tmps.tile([128, TOKC], F32, name="rope_t")
                            a = tmps.tile([128, TOKC], F32, name="rope_a")
                            nc.vector.tensor_tensor(
                                t[0:64], ps[64:128], sn_sb[0:64, c0:c0 + TOKC], ALU.mult)
                            nc.vector.tensor_tensor(
                                t[64:128], ps[0:64], sn_sb[64:128, c0:c0 + TOKC], ALU.mult)
                            nc.vector.tensor_tensor(
                                a, ps, cs_sb[:, c0:c0 + TOKC], ALU.mult)
                            nc.vector.tensor_tensor(
                                dsts[h][:, c0:c0 + TOKC], a, t, ALU.add)
                        else:
                            # V: copy back then PE-transpose to natural layout
                            vt = tmps.tile([128, TOKC], F32, name="vtmp")
                            nc.scalar.copy(vt, ps)
                            for s in range(TOKC // 128):
                                pt = psA.tile([128, 128], F32, name="ps_tr", tag="psA")
                                nc.tensor.transpose(pt, vt[:, s * 128:(s + 1) * 128], ident)
                                nc.vector.tensor_copy(
                                    vnat[h][:, (c0 // 128) + s, :], pt)

            # ---- attention + output projection, per 512-token q chunk ----
            for qc in range(NQC):
                q0 = qc * QC
                attn_sb = []
                for h in range(HPC):
                    pso = psO.tile([128, QC], F32, name="ps_out")
                    pss = psS.tile([1, QC], F32, name="ps_sum")
                    nkt = (qc + 1) * (QC // 128)
                    for kt in range(nkt):
                        psc = psA.tile([128, QC], F32, name="ps_sc", tag="psA")
                        nc.tensor.matmul(
                            psc, krot[h][:, kt * 128:(kt + 1) * 128],
                            qrot[h][:, q0:q0 + QC], start=True, stop=True,
                        )
                        e = esbp.tile([128, QC], F32R, name="e_sb")
                        nc.scalar.activation(e, psc, AF.Exp, scale=SCALE)
                        diag = kt - qc * (QC // 128)
                        if diag >= 0:
                            nc.vector.tensor_tensor(e, e, mk_sb[:, diag, :], ALU.mult)
                        nc.tensor.matmul(pso, vnat[h][:, kt, :], e,
                                         start=(kt == 0), stop=(kt == nkt - 1))
                        nc.tensor.matmul(pss, ones_col, e,
                                         start=(kt == 0), stop=(kt == nkt - 1))
                    rec = tmps.tile([1, QC], F32, name="recip")
                    nc.vector.reciprocal(rec, pss)
                    psb = psB.tile([128, QC], F32, name="ps_bc")
                    nc.tensor.matmul(psb, ones_row, rec, start=True, stop=True)
                    bc = bcp.tile([128, QC], F32, name="bc_sb")
                    nc.scalar.copy(bc, psb)
                    att = attnp.tile([128, QC], F32R, name=f"att{h}")
                    nc.vector.tensor_tensor(att, pso, bc, ALU.mult)
                    attn_sb.append(att)
                # w_out partial for these 512 tokens
                for mt in range(QC // 128):
                    t0 = q0 + mt * 128
                    for ec in range(D // 512):
                        psw = psA.tile([128, 512], F32, name="ps_w", tag="psA")
                        for h in range(HPC):
                            nc.tensor.matmul(
                                psw, attn_sb[h][:, mt * 128:(mt + 1) * 128],
                                wo_sb[:, h, ec * 512:(ec + 1) * 512],
                                start=(h == 0), stop=(h == HPC - 1),
                            )
                        ob = outp.tile([128, 512], F32, name="out_sb")
                        if (mt + ec) % 2 == 0:
                            nc.scalar.copy(ob, psw)
                        else:
                            nc.vector.tensor_copy(ob, psw)
                        nc.sync.dma_start(
                            out[b, t0:t0 + 128, ec * 512:(ec + 1) * 512], ob)


def build_kernel():
    nc = bacc.Bacc(
        "TRN2",
        target_bir_lowering=False,
        debug=False,
        enable_asserts=False,
        num_devices=NCORES,
    )
    xt = nc.dram_tensor("xt", [B, D, L], F32R, kind="ExternalInput").ap()
    wq = nc.dram_tensor("wq", [128, KD, HPC * DH], F32R, kind="ExternalInput").ap()
    wk = nc.dram_tensor("wk", [128, KD, HPC * DH], F32R, kind="ExternalInput").ap()
    wv = nc.dram_tensor("wv", [128, KD, HPC * DH], F32R, kind="ExternalInput").ap()
    wo = nc.dram_tensor("wo", [128, HPC, D], F32R, kind="ExternalInput").ap()
    cs = nc.dram_tensor("cs", [128, L], F32, kind="ExternalInput").ap()
    sn = nc.dram_tensor("sn", [128, L], F32, kind="ExternalInput").ap()
    mk = nc.dram_tensor("mk", [128, 4, QC], F32R, kind="ExternalInput").ap()
    out = nc.dram_tensor("out", [B, L, D], F32, kind="ExternalOutput").ap()

    with tile.TileContext(nc) as tc:
        _body(nc, tc, (xt, wq, wk, wv, wo, cs, sn, mk, out))
    nc.compile()
    return nc


def _rope_tables():
    inv_freq = (1.0 / (ROPE_BASE ** (np.arange(0, DH, 2, dtype=np.float32) / DH))
                ).astype(np.float32)
    freqs = (np.arange(L, dtype=np.float32)[:, None] * inv_freq[None, :]
             ).astype(np.float32)  # [L, 64]
    cos_t = np.cos(freqs).astype(np.float32).T  # [64, L]
    sin_t = np.sin(freqs).astype(np.float32).T
    cs = np.concatenate([cos_t, cos_t], axis=0)  # [128, L]
    sn = np.concatenate([-sin_t, sin_t], axis=0)
    return np.ascontiguousarray(cs), np.ascontiguousarray(sn)


def _host_inputs(x, w_qkv, w_out):
    xt = np.ascontiguousarray(np.transpose(x, (0, 2, 1)))  # [B, D, L]
    cs, sn = _rope_tables()
    p = np.arange(128)[:, None]
    f = np.arange(QC)[None, :]
    mk = np.stack(
        [((bi * 128 + p) <= f).astype(np.float32) for bi in range(4)], axis=1
    )  # [128, 4, 512]
    mk = np.ascontiguousarray(mk)

    def wtile(wT):  # [D, M] -> [128, D//128, M]
        return np.ascontiguousarray(
            wT.reshape(KD, 128, wT.shape[1]).transpose(1, 0, 2))

    in_maps = []
    for c in range(NCORES):
        r0 = c * HPC * DH
        r1 = r0 + HPC * DH
        wq_c = wtile(np.ascontiguousarray(w_qkv[r0:r1, :].T))
        wk_c = wtile(np.ascontiguousarray(w_qkv[D + r0:D + r1, :].T))
        wv_c = wtile(np.ascontiguousarray(w_qkv[2 * D + r0:2 * D + r1, :].T))
        wo_c = np.ascontiguousarray(
            w_out[:, r0:r1].T.reshape(HPC, 128, D).transpose(1, 0, 2))
        in_maps.append({
            "xt": xt, "wq": wq_c, "wk": wk_c, "wv": wv_c, "wo": wo_c,
            "cs": cs, "sn": sn, "mk": mk,
        })
    return in_maps


_NC_CACHE = []


def _get_nc():
    if not _NC_CACHE:
        _NC_CACHE.append(build_kernel())
    return _NC_CACHE[0]


def kernel(x, w_qkv, w_out):
    x = np.asarray(x, dtype=np.float32)
    w_qkv = np.asarray(w_qkv, dtype=np.float32)
    w_out = np.asarray(w_out, dtype=np.float32)
    nc = _get_nc()
    in_maps = _host_inputs(x, w_qkv, w_out)
    res = run_bass_kernel_spmd(nc, in_maps, core_ids=list(range(NCORES)))
    acc = res.results[0]["out"].astype(np.float32)
    for c in range(1, NCORES):
        acc += res.results[c]["out"]
    return acc


# revision 47
# speedup vs baseline: 28366.9819x; 28366.9819x over previous
"""Causal self-attention with RoPE, tensor-parallel over 8 NeuronCores.

Problem shapes: x [2, 2048, 2048], w_qkv [6144, 2048], w_out [2048, 2048],
H=16 heads, dh=128. Core c owns heads {2c, 2c+1}.

Per-core plan (all layouts chosen so no on-device transposes are needed
except V, which gets a cheap PE transpose):
  - host passes xT [B, D, L] plus per-core transposed weight shards
  - QKV^T projection: psum[dh, tok] = W^T-tile (stationary) x xT (moving)
  - RoPE applied during PSUM->SBUF copyback via crossed-base-partition
    DVE ops (half-swap without any shuffle instruction)
  - scores^T[k, q] = K^T-tile (stationary) x Q^T (moving); exp on ACT with
    the 1/sqrt(dh) scale folded in; causal mask = multiply by 0/1 tiles on
    the (only partially-valid) diagonal blocks; fully-masked blocks skipped
  - unnormalized out^T[dh, q] accumulates V-tile x E^T; the softmax
    denominator accumulates ones^T x E^T in the same pass
  - normalization: reciprocal of the [1, 512] sums row, broadcast across
    partitions with a K=1 matmul, multiplied in during the attention-out
    copyback
  - w_out partial[tok, e] = attnout^T-tile (stationary) x woT (moving),
    summed over this core's heads in PSUM; host sums the 8 partials

Matmuls run in float32r (full PE rate for free dim >= 256, ~1.5e-4 rel
err per K=128 contraction measured on hw).
"""

import numpy as np

import concourse.bass as bass
import concourse.mybir as mybir
import concourse.tile as tile
from concourse import bacc, library_config
from concourse.bass_utils import run_bass_kernel_spmd

B, L, D, H = 2, 2048, 2048, 16
DH = D // H  # 128
NCORES = 8
HPC = H // NCORES  # heads per core
ROPE_BASE = 10000.0
SCALE = 1.0 / float(np.sqrt(np.float32(DH)))

TOKC = 256  # token chunk width in the QKV projection phase
NCHUNK = L // TOKC  # 8
QC = 512  # q chunk width in the attention phase
NQC = L // QC  # 4
KT = L // 128  # 16 k tiles per sequence
KD = D // 128  # 16 contraction chunks for the projections

F32 = mybir.dt.float32
F32R = mybir.dt.float32r
AF = mybir.ActivationFunctionType
ALU = mybir.AluOpType


def _body(nc, tc, aps, phases=("qkv", "attn", "wout")):
    xt, wq, wk, wv, wo, cs, mk, idn, out = aps
    with (
        tc.tile_pool(name="const", bufs=1) as const,
        tc.tile_pool(name="xtp", bufs=2) as xtp,
        tc.tile_pool(name="qkv", bufs=1) as qkvp,
        tc.tile_pool(name="tmps", bufs=2) as tmps,
        tc.tile_pool(name="esb", bufs=3) as esbp,
        tc.tile_pool(name="bcp", bufs=2) as bcp,
        tc.tile_pool(name="attn", bufs=2) as attnp,
        tc.tile_pool(name="outp", bufs=3) as outp,
        tc.tile_pool(name="psA", bufs=4, space="PSUM") as psA,
        tc.tile_pool(name="psO", bufs=2, space="PSUM") as psO,
        tc.tile_pool(name="psS", bufs=2, space="PSUM") as psS,
    ):
        # ---- constants ----
        wq_sb = const.tile([128, KD, HPC * DH], F32R, name="wq_sb")
        wk_sb = const.tile([128, KD, HPC * DH], F32R, name="wk_sb")
        wv_sb = const.tile([128, KD, HPC * DH], F32R, name="wv_sb")
        wo_sb = const.tile([128, HPC, D], F32R, name="wo_sb")
        # tbl packs cos (partitions 0:64) and sin (partitions 64:128)
        tbl_sb = const.tile([128, L], F32, name="tbl_sb")
        mk_sb = const.tile([128, 4, QC], F32R, name="mk_sb")
        ident = const.tile([128, 128], F32, name="ident")

        def load_chunk(b, c):
            c0 = c * TOKC
            xtile = xtp.tile([128, KD, TOKC], F32R, name="xtile")
            src = xt[b, :, c0:c0 + TOKC].rearrange("(ko p) n -> p ko n", p=128)
            # two half-loads: matmuls on k<8 start as soon as half 0 lands
            nc.sync.dma_start(xtile[:, 0:KD // 2], src[:, 0:KD // 2])
            nc.sync.dma_start(xtile[:, KD // 2:], src[:, KD // 2:])
            return xtile

        # first x chunk + first weight ahead of everything else; wo last
        # (first needed ~150us in) so the first QKV matmul starts early
        xtile00 = load_chunk(0, 0) if "qkv" in phases else None
        for dst, src in ((wq_sb, wq), (tbl_sb, cs), (mk_sb, mk), (ident, idn),
                         (wk_sb, wk), (wv_sb, wv), (wo_sb, wo)):
            nc.sync.dma_start(dst, src)
        ones_f32 = const.tile([128, 1], F32, name="ones_f32")
        nc.vector.memset(ones_f32, 1.0)
        ones_col = const.tile([128, 1], F32R, name="ones_col")
        nc.vector.tensor_copy(ones_col, ones_f32)

        for b in range(B):
            # ---- QKV projection + RoPE for batch b ----
            qrot = [qkvp.tile([128, L], F32R, name=f"qrot{h}") for h in range(HPC)]
            krot = [qkvp.tile([128, L], F32R, name=f"krot{h}") for h in range(HPC)]
            vnat = [qkvp.tile([128, KT, 128], F32R, name=f"vnat{h}")
                    for h in range(HPC)]
            for c in range(NCHUNK if "qkv" in phases else 0):
                c0 = c * TOKC
                xtile = xtile00 if (b == 0 and c == 0) else load_chunk(b, c)
                for w_sb, dsts, kind in (
                    (wq_sb, qrot, "q"), (wk_sb, krot, "k"), (wv_sb, vnat, "v")
                ):
                    for h in range(HPC):
                        ps = psA.tile([128, TOKC], F32, name="ps_proj", tag="psA")
                        for k in range(KD):
                            nc.tensor.matmul(
                                ps, w_sb[:, k, h * DH:(h + 1) * DH], xtile[:, k, :],
                                start=(k == 0), stop=(k == KD - 1),
                            )
                        if "nocopy" in phases:
                            continue
                        if kind in ("q", "k"):
                            # RoPE copyback: dst = psum*cos +/- swap(psum)*sin
                            # tbl[0:64] = cosT, tbl[64:128] = sinT (64 freq rows)
                            cseg = tbl_sb[0:64, c0:c0 + TOKC]
                            sseg = tbl_sb[64:128, c0:c0 + TOKC]
                            t = tmps.tile([128, TOKC], F32, name="rope_t")
                            a = tmps.tile([128, TOKC], F32, name="rope_a")
                            nc.vector.tensor_tensor(t[0:64], ps[64:128], sseg, ALU.mult)
                            nc.vector.tensor_tensor(t[64:128], ps[0:64], sseg, ALU.mult)
                            nc.vector.tensor_tensor(a[0:64], ps[0:64], cseg, ALU.mult)
                            nc.vector.tensor_tensor(a[64:128], ps[64:128], cseg, ALU.mult)
                            dst = dsts[h]
                            nc.vector.tensor_tensor(
                                dst[0:64, c0:c0 + TOKC], a[0:64], t[0:64], ALU.subtract)
                            nc.vector.tensor_tensor(
                                dst[64:128, c0:c0 + TOKC], a[64:128], t[64:128], ALU.add)
                        else:
                            # V: copy back then PE-transpose to natural layout
                            vt = tmps.tile([128, TOKC], F32, name="vtmp")
                            nc.vector.tensor_copy(vt, ps)
                            for s in range(TOKC // 128):
                                pt = psA.tile([128, 128], F32, name="ps_tr", tag="psA")
                                nc.tensor.transpose(pt, vt[:, s * 128:(s + 1) * 128], ident)
                                nc.vector.tensor_copy(
                                    vnat[h][:, (c0 // 128) + s, :], pt)

            # ---- attention + output projection, per 512-token q chunk ----
            # w_out for chunk qc is emitted after attention for qc+1 so the
            # PE never waits on the softmax-normalize chain.
            def emit_wout(attn_sb, q0):
                for mt in range(QC // 128 if "wout" in phases else 0):
                    t0 = q0 + mt * 128
                    for ec in range(D // 512):
                        psw = psA.tile([128, 512], F32, name="ps_w", tag="psA")
                        for h in range(HPC):
                            nc.tensor.matmul(
                                psw, attn_sb[h][:, mt * 128:(mt + 1) * 128],
                                wo_sb[:, h, ec * 512:(ec + 1) * 512],
                                start=(h == 0), stop=(h == HPC - 1),
                            )
                        ob = outp.tile([128, 512], F32, name="out_sb")
                        if (mt + ec) % 2 == 0:
                            nc.scalar.copy(ob, psw)
                        else:
                            nc.vector.tensor_copy(ob, psw)
                        eng = nc.scalar if (mt + ec) % 2 == 0 else nc.sync
                        eng.dma_start(
                            out[b, t0:t0 + 128, ec * 512:(ec + 1) * 512], ob)

            pend = None
            for qc in range(NQC if "attn" in phases else 0):
                q0 = qc * QC
                attn_sb = []
                for h in range(HPC):
                    pso = psO.tile([128, QC], F32, name="ps_out")
                    pss = psS.tile([1, QC], F32, name="ps_sum")
                    nkt = (qc + 1) * (QC // 128)
                    for kt in range(nkt):
                        psc = psA.tile([128, QC], F32, name="ps_sc", tag="psA")
                        nc.tensor.matmul(
                            psc, krot[h][:, kt * 128:(kt + 1) * 128],
                            qrot[h][:, q0:q0 + QC], start=True, stop=True,
                        )
                        e = esbp.tile([128, QC], F32R, name="e_sb")
                        if "noexp" in phases:
                            nc.scalar.copy(e, psc)
                        else:
                            nc.scalar.activation(e, psc, AF.Exp, scale=SCALE)
                        diag = kt - qc * (QC // 128)
                        if diag >= 0 and "nomask" not in phases:
                            nc.vector.tensor_tensor(e, e, mk_sb[:, diag, :], ALU.mult)
                        nc.tensor.matmul(pso, vnat[h][:, kt, :], e,
                                         start=(kt == 0), stop=(kt == nkt - 1))
                        if "nosum" not in phases:
                            nc.tensor.matmul(pss, ones_col, e,
                                             start=(kt == 0), stop=(kt == nkt - 1))
                    att = attnp.tile([128, QC], F32R, name=f"att{h}")
                    if "nosum" in phases:
                        nc.vector.tensor_copy(att, pso)
                    else:
                        rec = tmps.tile([1, QC], F32, name="recip")
                        nc.vector.reciprocal(rec, pss)
                        bc = bcp.tile([128, QC], F32, name="bc_sb")
                        nc.gpsimd.partition_broadcast(bc, rec)
                        nc.vector.tensor_tensor(att, pso, bc, ALU.mult)
                    attn_sb.append(att)
                if pend is not None:
                    emit_wout(*pend)
                pend = (attn_sb, q0)
            if pend is not None:
                emit_wout(*pend)


def build_kernel(timing=False, loop_n=0, phases=("qkv", "attn", "wout")):
    nc = bacc.Bacc(
        "TRN2",
        target_bir_lowering=False,
        debug=False,
        enable_asserts=False,
        num_devices=NCORES,
    )
    xt = nc.dram_tensor("xt", [B, D, L], F32R, kind="ExternalInput").ap()
    wq = nc.dram_tensor("wq", [128, KD, HPC * DH], F32R, kind="ExternalInput").ap()
    wk = nc.dram_tensor("wk", [128, KD, HPC * DH], F32R, kind="ExternalInput").ap()
    wv = nc.dram_tensor("wv", [128, KD, HPC * DH], F32R, kind="ExternalInput").ap()
    wo = nc.dram_tensor("wo", [128, HPC, D], F32R, kind="ExternalInput").ap()
    cs = nc.dram_tensor("cs", [128, L], F32, kind="ExternalInput").ap()
    mk = nc.dram_tensor("mk", [128, 4, QC], F32R, kind="ExternalInput").ap()
    idn = nc.dram_tensor("idn", [128, 128], F32, kind="ExternalInput").ap()
    out_kind = "Internal" if timing else "ExternalOutput"
    out = nc.dram_tensor("out", [B, L, D], F32, kind=out_kind).ap()
    done = None
    if timing:
        done = nc.dram_tensor("done", [1, 4], F32, kind="ExternalOutput").ap()

    nc.gpsimd.load_library(library_config.attn)
    aps = (xt, wq, wk, wv, wo, cs, mk, idn, out)
    with tile.TileContext(nc) as tc:
        if loop_n:
            with tc.For_i(0, loop_n, 1):
                _body(nc, tc, aps, phases)
        else:
            _body(nc, tc, aps, phases)
        if timing:
            # tiny output so the executable has an ExternalOutput; depends on
            # one real out tile via a DRAM->DRAM DMA of the last row.
            nc.sync.dma_start(done, out[B - 1, L - 1:L, 0:4])
    nc.compile()
    return nc


def _rope_tables():
    inv_freq = (1.0 / (ROPE_BASE ** (np.arange(0, DH, 2, dtype=np.float32) / DH))
                ).astype(np.float32)
    freqs = (np.arange(L, dtype=np.float32)[:, None] * inv_freq[None, :]
             ).astype(np.float32)  # [L, 64]
    cos_t = np.cos(freqs).astype(np.float32).T  # [64, L]
    sin_t = np.sin(freqs).astype(np.float32).T
    tbl = np.concatenate([cos_t, sin_t], axis=0)  # [128, L]
    return np.ascontiguousarray(tbl)


def _host_inputs(x, w_qkv, w_out):
    xt = np.ascontiguousarray(np.transpose(x, (0, 2, 1)))  # [B, D, L]
    cs = _rope_tables()
    p = np.arange(128)[:, None]
    f = np.arange(QC)[None, :]
    mk = np.stack(
        [((bi * 128 + p) <= f).astype(np.float32) for bi in range(4)], axis=1
    )  # [128, 4, 512]
    mk = np.ascontiguousarray(mk)

    def wtile(wT):  # [D, M] -> [128, D//128, M]
        return np.ascontiguousarray(
            wT.reshape(KD, 128, wT.shape[1]).transpose(1, 0, 2))

    in_maps = []
    for c in range(NCORES):
        r0 = c * HPC * DH
        r1 = r0 + HPC * DH
        wq_c = wtile(np.ascontiguousarray(w_qkv[r0:r1, :].T))
        wk_c = wtile(np.ascontiguousarray(w_qkv[D + r0:D + r1, :].T))
        wv_c = wtile(np.ascontiguousarray(w_qkv[2 * D + r0:2 * D + r1, :].T))
        wo_c = np.ascontiguousarray(
            w_out[:, r0:r1].T.reshape(HPC, 128, D).transpose(1, 0, 2))
        in_maps.append({
            "xt": xt, "wq": wq_c, "wk": wk_c, "wv": wv_c, "wo": wo_c,
            "cs": cs, "mk": mk, "idn": np.eye(128, dtype=np.float32),
        })
    return in_maps


_NC_CACHE = []


def _get_nc():
    if not _NC_CACHE:
        _NC_CACHE.append(build_kernel())
    return _NC_CACHE[0]


def kernel(x, w_qkv, w_out):
    x = np.asarray(x, dtype=np.float32)
    w_qkv = np.asarray(w_qkv, dtype=np.float32)
    w_out = np.asarray(w_out, dtype=np.float32)
    nc = _get_nc()
    in_maps = _host_inputs(x, w_qkv, w_out)
    res = run_bass_kernel_spmd(nc, in_maps, core_ids=list(range(NCORES)))
    acc = res.results[0]["out"].astype(np.float32)
    for c in range(1, NCORES):
        acc += res.results[c]["out"]
    return acc
